# revision 5
# baseline (speedup 1.0000x reference)
"""BlockGRUCell Trainium2 kernel.

Computation (per reference):
  hx = concat([h, x], -1)                       # (B, 2048)
  gate[b, 192g+o] = sum_i hx[b, 128g+i] * W[g, o, i]   # block-diagonal matmul
  r, c, u = split(gate + bias, 3)               # bias == 0 from setup_inputs
  h_new = sigmoid(u) * tanh(sigmoid(r) * c) + (1 - sigmoid(u)) * h

Sharding: data-parallel over batch across 8 NeuronCores (2048 rows each),
weights replicated.

The TensorE matmul contracts over the partition dim, so the stationary
operand must be hx^T per 128-feature block. The host pre-packs hx into
per-tile transposed fp8(e4m3) panels:
  hxt[t, p, 128g+b] = hx[128t+b, 128g+p]

Engine budget per core (measured):
  - ACT and DVE are co-bottlenecks (~6us per tile pair, ~95% interleaved).
    3 LUT passes/element are irreducible; sigmoid(r)/sigmoid(u) must read
    f32 PSUM panels per tile (8-bank PSUM ring can't hold pairs), but
    tanh reads rc from SBUF and the blend chain is SBUF bf16, so those
    run QUAD-wide (FD=4096) to amortize the ~150-250ns per-instruction
    bubble: tanh(rc4), dd4 = cand4-h4, ee4 = upd4*dd4, out4 = h4+ee4.
  - bf16 SBUF step-1 tensor_tensor runs 2x_1P; fp32/PSUM runs 1x, so
    only rc = gC(PSUM)*reset stays per-tile fp32-read.
  - h / out move as bf16 pairs; hxt in fp8 (tolerance 2e-2: bf16 blend
    adds ~3e-3, fp8 hx adds ~5e-3).
  - Ramp: tile 0 runs gate-major matmuls (all r columns first) with a
    half-split epilogue, wt loads by gate thirds (r first, c/u after
    tile 0's hxt), so the first sigmoid fires as early as possible.
"""

import numpy as np
import ml_dtypes

import concourse.bass as bass
import concourse.bacc as bacc
import concourse.tile as tile
import concourse.mybir as mybir
from concourse.bass_utils import run_bass_kernel_spmd

N_CORES = 8
BATCH = 16384
BS = BATCH // N_CORES            # rows per core
P = 128
NT = BS // P                     # 128-row tiles per core
HID = 1024
G = 16                           # feature blocks
IN_PER = 128
OUT_PER = 192
GATE = 3 * HID                   # 3072
PSUM_BANK_F32 = 512

F32 = mybir.dt.float32
BF16 = mybir.dt.bfloat16
FP8 = mybir.dt.float8e4
AFT = mybir.ActivationFunctionType
MUL = mybir.AluOpType.mult

HXT_FP8 = True                   # hx^T panels in fp8 e4m3 (halves hxt DMA)
HXT_DT = FP8 if HXT_FP8 else BF16
HXT_NP = ml_dtypes.float8_e4m3 if HXT_FP8 else ml_dtypes.bfloat16

# epilogue groups: (first tile, #tiles, #column-splits per tile).
# small groups at the edges keep the serial ACT<->DVE chain fine-grained
# during ramp-in/drain; quads in the middle amortize per-op overhead.
GROUPS = [(0, 1, 2), (1, 1, 1), (2, 2, 1),
          (4, 4, 1), (8, 4, 1),
          (12, 2, 1), (14, 1, 1), (15, 1, 2)]


def _mm_splits(block_major):
    """[(c0, c1, g)] matmul column splits at PSUM bank boundaries."""
    out = []
    for g in range(G):
        c0 = g * OUT_PER
        while c0 < (g + 1) * OUT_PER:
            c1 = min((g + 1) * OUT_PER,
                     (c0 // PSUM_BANK_F32 + 1) * PSUM_BANK_F32)
            out.append((c0, c1, g))
            c0 = c1
    if not block_major:
        # gate-major: all r-gate columns first so sigmoid(r) starts as
        # soon as the first thirds of wt/hxt land (ramp-in only; costs
        # extra LDWEIGHTS so not used steady-state)
        out.sort(key=lambda s: s[0])
    return out


def _body(tc, nc, hxt_d, h_d, wt_d, out_d):
    with (
        tc.tile_pool(name="consts", bufs=1) as consts,
        tc.tile_pool(name="io", bufs=6) as io,
        tc.tile_pool(name="panels", bufs=4) as panels,
        tc.tile_pool(name="quads", bufs=2) as quads,
        tc.tile_pool(name="gatep", bufs=4, space="PSUM") as gatep,
    ):
        # warm the sigmoid/tanh ACT table during the initial DMAs (the
        # ~2.7us ACT_TABLE_LOAD otherwise lands on tile 0's critical path)
        warm = consts.tile([P, 1], F32)
        nc.vector.memset(warm, 0.0)
        nc.scalar.activation(warm, warm, AFT.Sigmoid)

        # only the r third of wt gates tile 0's first sigmoid; c/u thirds
        # are emitted after tile 0's hxt so they don't delay it
        wt_s = consts.tile([P, G * OUT_PER], BF16)
        nc.sync.dma_start(out=wt_s[:, 0:HID], in_=wt_d[:, 0:HID])

        groups = {g[0]: g for g in GROUPS}
        hQ = outQ = rcQ = updQ = None
        g0 = n_g = 0
        for t in range(NT):
            if t in groups:
                g0, n_g, n_spl = groups[t]
            q, half = divmod(t, 2)

            hxt = io.tile([P, G * P], HXT_DT, tag="hxt")
            if t == 0:
                nc.sync.dma_start(out=hxt[:, 0:G * P // 2],
                                  in_=hxt_d[0, :, 0:G * P // 2])
                nc.sync.dma_start(out=hxt[:, G * P // 2:],
                                  in_=hxt_d[0, :, G * P // 2:])
                nc.sync.dma_start(out=wt_s[:, HID:2 * HID],
                                  in_=wt_d[:, HID:2 * HID])
                nc.sync.dma_start(out=wt_s[:, 2 * HID:],
                                  in_=wt_d[:, 2 * HID:])
            else:
                nc.sync.dma_start(out=hxt, in_=hxt_d[t])

            if t % 4 == 0:
                # quad-wide SBUF buffers; h arrives as two pair-packed
                # bf16 DMAs (the first is deferred below the matmul feeds)
                hQ = io.tile([P, 4 * HID], BF16, tag="hQ", bufs=2)
                outQ = io.tile([P, 4 * HID], BF16, tag="outQ", bufs=2)
                rcQ = quads.tile([P, 4 * HID], BF16, tag="rcQ")
                updQ = quads.tile([P, 4 * HID], BF16, tag="updQ")
                if t > 0:
                    nc.sync.dma_start(out=hQ[:, 0:2 * HID], in_=h_d[t // 2])
            if half == 0 and t % 4 == 2:
                nc.sync.dma_start(out=hQ[:, 2 * HID:], in_=h_d[q])
            tq = t % 4                      # tile index within the quad

            # gate panels = the r/c/u split exactly (2 PSUM banks each)
            gR = gatep.tile([P, HID], F32, tag="gate")
            gC = gatep.tile([P, HID], F32, tag="gate")
            gU = gatep.tile([P, HID], F32, tag="gate")
            gs = (gR, gC, gU)

            for c0, c1, g in _mm_splits(block_major=(t > 0)):
                gate = gs[c0 // HID]
                nc.tensor.matmul(gate[:, c0 % HID:(c0 % HID) + c1 - c0],
                                 hxt[:, g * P:(g + 1) * P], wt_s[:, c0:c1],
                                 start=True, stop=True)

            if t == 0:
                nc.sync.dma_start(out=hQ[:, 0:2 * HID], in_=h_d[0])

            reset = panels.tile([P, HID], F32, tag="reset")
            rc_t = rcQ[:, tq * HID:(tq + 1) * HID]
            upd_t = updQ[:, tq * HID:(tq + 1) * HID]

            ns = groups[t][2] if t in groups else 1
            w = HID // ns
            for i in range(ns):
                a, b = i * w, (i + 1) * w
                nc.scalar.activation(reset[:, a:b], gR[:, a:b], AFT.Sigmoid)
                nc.vector.tensor_tensor(rc_t[:, a:b], gC[:, a:b],
                                        reset[:, a:b], MUL)
                nc.scalar.activation(upd_t[:, a:b], gU[:, a:b], AFT.Sigmoid)

            if t == g0 + n_g - 1:
                # group-wide epilogue over tiles [g0, g0+n_g)
                lo = (g0 % 4) * HID
                hi = lo + n_g * HID
                candG = quads.tile([P, 4 * HID], BF16, tag="candG")
                ddG = quads.tile([P, 4 * HID], BF16, tag="ddG")
                eeG = quads.tile([P, 4 * HID], BF16, tag="eeG")
                gw = (hi - lo) // ns
                qbase = (t // 4) * 2
                for i in range(ns):
                    a, b = lo + i * gw, lo + (i + 1) * gw
                    nc.scalar.activation(candG[:, a:b], rcQ[:, a:b],
                                         AFT.Tanh)
                    nc.vector.tensor_sub(ddG[:, a:b], candG[:, a:b],
                                         hQ[:, a:b])
                    nc.vector.tensor_mul(eeG[:, a:b], updQ[:, a:b],
                                         ddG[:, a:b])
                    nc.vector.tensor_add(outQ[:, a:b], hQ[:, a:b],
                                         eeG[:, a:b])
                    if t == NT - 1:
                        # stream the final tile's stores per split
                        pp = a // (2 * HID)
                        nc.sync.dma_start(
                            out=out_d[qbase + pp][:, a % (2 * HID):
                                                  a % (2 * HID) + gw],
                            in_=outQ[:, a:b])
                if t != NT - 1:
                    # store the group's columns in pair-aligned chunks
                    for pp in range(lo // (2 * HID),
                                    (hi + 2 * HID - 1) // (2 * HID)):
                        sa = max(lo, pp * 2 * HID)
                        sb = min(hi, (pp + 1) * 2 * HID)
                        nc.sync.dma_start(
                            out=out_d[qbase + pp][:, sa % (2 * HID):
                                                  sa % (2 * HID) + sb - sa],
                            in_=outQ[:, sa:sb])


_NC_CACHE = {}


def _build_nc():
    if "nc" in _NC_CACHE:
        return _NC_CACHE["nc"]
    nc = bacc.Bacc()
    hxt_d = nc.dram_tensor("hxt", [NT, P, G * P], HXT_DT, kind="ExternalInput")
    h_d = nc.dram_tensor("h2", [NT // 2, P, 2 * HID], BF16,
                         kind="ExternalInput")
    wt_d = nc.dram_tensor("wt", [P, G * OUT_PER], BF16, kind="ExternalInput")
    out_d = nc.dram_tensor("out", [NT // 2, P, 2 * HID], BF16,
                           kind="ExternalOutput")
    with tile.TileContext(nc) as tc:
        _body(tc, nc, hxt_d, h_d, wt_d, out_d)
    nc.compile()
    _NC_CACHE["nc"] = nc
    return nc


def _np_reference(x, h, weight, bias):
    hx = np.concatenate([h, x], axis=-1)
    xg = hx.reshape(x.shape[0], G, IN_PER)
    gate = np.einsum("bgi,goi->bgo", xg, weight).reshape(x.shape[0], GATE)
    gate = gate + bias
    r, c, u = np.split(gate, 3, axis=-1)
    reset = 1.0 / (1.0 + np.exp(-r))
    cand = np.tanh(reset * c)
    upd = 1.0 / (1.0 + np.exp(-u))
    return (upd * cand + (1.0 - upd) * h).astype(np.float32)


def _pack_hxt(hs, xs):
    """-> [NT, 128, 2048] with hxt[t, p, 128g+b] = hx[128t+b, 128g+p],
    where hx = concat([h, x], -1) per-row (blocks 0-7 = h, 8-15 = x)."""
    def tp(a):                      # [BS, 1024] -> [NT, 128, 8, 128]
        return a.reshape(NT, P, 8, P).transpose(0, 3, 2, 1)   # [t, p, g, b]
    arr = np.concatenate([tp(hs), tp(xs)], axis=2)            # [t, p, 16, b]
    return np.ascontiguousarray(arr.reshape(NT, P, G * P)).astype(HXT_NP)


def _pack_pairs(a):
    """[BS, 1024] -> [NT//2, 128, 2048] bf16 with
    [q, p, 1024s+f] = a[256q+128s+p, f]."""
    return np.ascontiguousarray(
        a.reshape(NT // 2, 2, P, HID).transpose(0, 2, 1, 3)
        .reshape(NT // 2, P, 2 * HID)).astype(ml_dtypes.bfloat16)


def _unpack_pairs(a):
    """inverse of _pack_pairs, upcast to fp32."""
    return np.ascontiguousarray(
        a.reshape(NT // 2, P, 2, HID).transpose(0, 2, 1, 3)
        .reshape(BS, HID)).astype(np.float32)


def _run(x, h, weight, bias, trace=False, tmpdir=None):
    # wt[p, 192g+o] = W[g, o, p] — the exact SBUF layout, one contiguous DMA
    wt = np.ascontiguousarray(
        weight.transpose(2, 0, 1).reshape(P, G * OUT_PER)).astype(
        ml_dtypes.bfloat16)
    nc = _build_nc()
    in_maps = []
    for c in range(N_CORES):
        sl = slice(c * BS, (c + 1) * BS)
        xs, hs = x[sl], h[sl]
        in_maps.append({
            "hxt": _pack_hxt(hs, xs),
            "h2": _pack_pairs(hs),
            "wt": wt,
        })
    res = run_bass_kernel_spmd(nc, in_maps, core_ids=list(range(N_CORES)),
                               trace=trace, tmpdir=tmpdir)
    out = np.concatenate([_unpack_pairs(m["out"]) for m in res.results],
                         axis=0)
    return out, res


def kernel(x, h, weight, bias):
    x = np.asarray(x, dtype=np.float32)
    h = np.asarray(h, dtype=np.float32)
    weight = np.asarray(weight, dtype=np.float32)
    bias = np.asarray(bias, dtype=np.float32)
    if np.any(bias != 0.0):
        # setup_inputs() always passes zero bias; keep a correct fallback.
        return _np_reference(x, h, weight, bias)
    out, _ = _run(x, h, weight, bias)
    return out
